# revision 1
# baseline (speedup 1.0000x reference)
"""LSTM-style scan (named GRU) Trainium2 Bass kernel.

Problem: x [64, 256, 1024], W [2048, 768], b [2048] -> y [64, 512, 1024]
  per step t: fea = concat([x_t, h]) @ W.T + b ; i,j,f,o = split(fea, 4)
  c = c*sig(f) + sig(i)*tanh(j) ; h = tanh(c)*sig(o); y[:, :, t] = h

Strategy (8 NeuronCores, data-parallel over batch, 8 rows/core):
- Everything runs transposed: gates/c_out on SBUF partitions, batch on the
  free dim, so per-step activations are [128, 32] tiles and h.T feeds the
  next matmul directly (no per-step transpose).
- Phase 1 (parallel over T): pre.T = Wx_perm @ x.T + b as one big matmul,
  streamed to a DRAM scratch buffer in bf16.
- Phase 2 (sequential scan): per step, fea.T chunks = sum_k WhT[k].T @ h.T
  with stationary bf16 weight tiles (fast weight load), accumulated in
  PSUM; pre added on VectorE; sig/tanh on ScalarE.
- Gate rows are host-permuted to [i, f, j, o] so sigmoid(i,f) is one
  contiguous activation op.
"""

import numpy as np
import ml_dtypes

B, C_IN, C_OUT, T_FULL = 64, 256, 512, 1024
N_CORES = 8
B_LOC = B // N_CORES  # 8
G = 4 * C_OUT  # 2048
NM = G // 128  # 16 gate chunks
NKH = C_OUT // 128  # 4 h chunks
NKX = C_IN // 128  # 2 x chunks
TB = 64  # steps per scan block

_PROG_CACHE = {}


def _build_program(T):
    from contextlib import ExitStack

    import concourse.bass as bass
    import concourse.tile as tile
    from concourse import bacc, mybir

    FP32 = mybir.dt.float32
    BF16 = mybir.dt.bfloat16
    AF = mybir.ActivationFunctionType

    nc = bacc.Bacc(None, target_bir_lowering=False)

    xT = nc.dram_tensor("xT", [C_IN, T * B_LOC], FP32, kind="ExternalInput")
    wxT = nc.dram_tensor("wxT", [C_IN, G], FP32, kind="ExternalInput")
    whT = nc.dram_tensor("whT", [C_OUT, G], BF16, kind="ExternalInput")
    bmat = nc.dram_tensor("bmat", [128, NM], FP32, kind="ExternalInput")
    y_d = nc.dram_tensor("y", [128, T, NKH * B_LOC], BF16, kind="ExternalOutput")

    NB = T // TB  # blocks (phase-1 block == scan block == 64 steps)
    BC = TB * B_LOC  # free-dim columns per block (512)

    with ExitStack() as ctx:
        tc = ctx.enter_context(tile.TileContext(nc))
        static = ctx.enter_context(tc.tile_pool(name="static", bufs=1))
        xpool = ctx.enter_context(tc.tile_pool(name="xin", bufs=3))
        prepool = ctx.enter_context(tc.tile_pool(name="preout", bufs=4))
        psum1 = ctx.enter_context(tc.tile_pool(name="psum1", bufs=2, space="PSUM"))
        prescan = ctx.enter_context(tc.tile_pool(name="prescan", bufs=2))
        ypool = ctx.enter_context(tc.tile_pool(name="ystore", bufs=2))
        ps_if_pool = ctx.enter_context(tc.tile_pool(name="ps_if", bufs=2, space="PSUM"))
        ps_j_pool = ctx.enter_context(tc.tile_pool(name="ps_j", bufs=2, space="PSUM"))
        ps_o_pool = ctx.enter_context(tc.tile_pool(name="ps_o", bufs=2, space="PSUM"))
        tpool = ctx.enter_context(tc.tile_pool(name="tmps", bufs=3))
        cpool = ctx.enter_context(tc.tile_pool(name="cstate", bufs=2))

        # --- static weights into SBUF ---
        # PE matmuls may carry at most ONE sync wait through walrus codegen,
        # so every tile a matmul reads is laundered through a VectorE copy:
        # PE then only ever waits on the DVE semaphore.
        wx_sb = []
        for k in range(NKX):
            st = static.tile([128, G], FP32, tag=f"wxs{k}")
            nc.gpsimd.dma_start(st[:], wxT[k * 128 : (k + 1) * 128, :])
            t = static.tile([128, G], FP32, tag=f"wx{k}")
            nc.vector.tensor_copy(t[:], st[:])
            wx_sb.append(t)
        wh_sb = []
        for k in range(NKH):
            st = static.tile([128, G], BF16, tag=f"whs{k}")
            nc.gpsimd.dma_start(st[:], whT[k * 128 : (k + 1) * 128, :])
            t = static.tile([128, G], BF16, tag=f"wh{k}")
            nc.vector.tensor_copy(t[:], st[:])
            wh_sb.append(t)
        b_st = static.tile([128, NM], FP32, tag="biass")
        nc.gpsimd.dma_start(b_st[:], bmat[:, :])
        b_sb = static.tile([128, NM], FP32, tag="bias")
        nc.vector.tensor_copy(b_sb[:], b_st[:])

        h_raw = static.tile([128, 4 * B_LOC], BF16, tag="hraw")
        nc.gpsimd.memset(h_raw[:], 0.0)
        h_init = static.tile([128, 4 * B_LOC], BF16, tag="hinit")
        nc.vector.tensor_copy(h_init[:], h_raw[:])
        c_init = static.tile([128, 4 * B_LOC], FP32, tag="cinit")
        nc.gpsimd.memset(c_init[:], 0.0)

        # --- fused per-block: phase 1 (input projection) then the scan ---
        prev_h = h_init  # AP source tile holding h_{t-1}.T as [128, 4*B_LOC]
        prev_h_off = 0
        prev_c = c_init
        for blk in range(NB):
            c0 = blk * BC
            xin = []
            for k in range(NKX):
                st = xpool.tile([128, BC], FP32, tag=f"xins{k}")
                nc.gpsimd.dma_start(st[:], xT[k * 128 : (k + 1) * 128, c0 : c0 + BC])
                t = xpool.tile([128, BC], FP32, tag=f"xin{k}")
                nc.vector.tensor_copy(t[:], st[:])
                xin.append(t)
            pre_sb = prescan.tile([128, NM * BC], BF16, tag="pre_sb")
            for m in range(NM):
                ps = psum1.tile([128, BC], FP32, tag="ps1")
                for k in range(NKX):
                    nc.tensor.matmul(
                        ps[:],
                        wx_sb[k][:, m * 128 : (m + 1) * 128],
                        xin[k][:],
                        start=(k == 0),
                        stop=(k == NKX - 1),
                    )
                nc.vector.tensor_scalar_add(
                    pre_sb[:, m * BC : (m + 1) * BC], ps[:], b_sb[:, m : m + 1]
                )
            pre3 = pre_sb[:].rearrange("p (m c) -> p m c", m=NM)
            ystore = ypool.tile([128, TB * 4 * B_LOC], BF16, tag="ystore")

            for s in range(TB):
                so = s * B_LOC  # column offset of step s within block (pre)
                # matmuls: fea.T += WhT[k].T @ h.T, gate chunks i(0-3) f(4-7)
                # j(8-11) o(12-15) into three PSUM tiles (separate banks so
                # VectorE can read i/f while PE still writes j/o).
                ps_if = ps_if_pool.tile([128, 8 * B_LOC], FP32, tag="ps_if")
                ps_j = ps_j_pool.tile([128, 4 * B_LOC], FP32, tag="ps_j")
                ps_o = ps_o_pool.tile([128, 4 * B_LOC], FP32, tag="ps_o")
                for m in range(NM):
                    if m < 8:
                        out_ap = ps_if[:, m * B_LOC : (m + 1) * B_LOC]
                    elif m < 12:
                        out_ap = ps_j[:, (m - 8) * B_LOC : (m - 7) * B_LOC]
                    else:
                        out_ap = ps_o[:, (m - 12) * B_LOC : (m - 11) * B_LOC]
                    for k in range(NKH):
                        rhs = prev_h[
                            :, prev_h_off + k * B_LOC : prev_h_off + (k + 1) * B_LOC
                        ]
                        nc.tensor.matmul(
                            out_ap,
                            wh_sb[k][:, m * 128 : (m + 1) * 128],
                            rhs,
                            start=(k == 0),
                            stop=(k == NKH - 1),
                        )

                # activations (all [128, 32]-ish tiles; batch on free dim)
                fea_if = tpool.tile([128, 8 * B_LOC], FP32, tag="fea_if")
                nc.vector.tensor_add(
                    fea_if[:].rearrange("p (m c) -> p m c", m=8),
                    ps_if[:].rearrange("p (m c) -> p m c", m=8),
                    pre3[:, 0:8, so : so + B_LOC],
                )
                sig_if = tpool.tile([128, 8 * B_LOC], FP32, tag="sig_if")
                nc.scalar.activation(sig_if[:], fea_if[:], AF.Sigmoid)

                fea_j = tpool.tile([128, 4 * B_LOC], FP32, tag="fea_j")
                nc.vector.tensor_add(
                    fea_j[:].rearrange("p (m c) -> p m c", m=4),
                    ps_j[:].rearrange("p (m c) -> p m c", m=4),
                    pre3[:, 8:12, so : so + B_LOC],
                )
                tanh_j = tpool.tile([128, 4 * B_LOC], FP32, tag="tanh_j")
                nc.scalar.activation(tanh_j[:], fea_j[:], AF.Tanh)

                t1 = tpool.tile([128, 4 * B_LOC], FP32, tag="t1")
                nc.vector.tensor_mul(t1[:], sig_if[:, 0 : 4 * B_LOC], tanh_j[:])
                c_new = cpool.tile([128, 4 * B_LOC], FP32, tag="c")
                nc.vector.tensor_mul(
                    c_new[:], prev_c[:], sig_if[:, 4 * B_LOC : 8 * B_LOC]
                )
                nc.vector.tensor_add(c_new[:], c_new[:], t1[:])
                tanh_c = tpool.tile([128, 4 * B_LOC], FP32, tag="tanh_c")
                nc.scalar.activation(tanh_c[:], c_new[:], AF.Tanh)

                fea_o = tpool.tile([128, 4 * B_LOC], FP32, tag="fea_o")
                nc.vector.tensor_add(
                    fea_o[:].rearrange("p (m c) -> p m c", m=4),
                    ps_o[:].rearrange("p (m c) -> p m c", m=4),
                    pre3[:, 12:16, so : so + B_LOC],
                )
                sig_o = tpool.tile([128, 4 * B_LOC], FP32, tag="sig_o")
                nc.scalar.activation(sig_o[:], fea_o[:], AF.Sigmoid)

                yo = s * 4 * B_LOC
                nc.vector.tensor_mul(
                    ystore[:, yo : yo + 4 * B_LOC], tanh_c[:], sig_o[:]
                )

                prev_h = ystore
                prev_h_off = yo
                prev_c = c_new

            # flush this block's h outputs: y[cc, p, t0+s, b]
            # single contiguous DMA for the whole block so ystore slot
            # release costs one DMA-lane wait
            nc.gpsimd.dma_start(
                y_d[:, blk * TB : (blk + 1) * TB, :],
                ystore[:].rearrange("p (s cb) -> p s cb", s=TB),
            )

    nc.compile()
    return nc


def _get_program(T):
    if T not in _PROG_CACHE:
        _PROG_CACHE[T] = _build_program(T)
    return _PROG_CACHE[T]


def _prep_inputs(x, W, b, T):
    perm = np.concatenate(
        [
            np.arange(0, C_OUT),  # i
            np.arange(2 * C_OUT, 3 * C_OUT),  # f
            np.arange(C_OUT, 2 * C_OUT),  # j
            np.arange(3 * C_OUT, 4 * C_OUT),  # o
        ]
    )
    Wp = np.asarray(W, dtype=np.float32)[perm]
    wxT = np.ascontiguousarray(Wp[:, :C_IN].T)
    whT = np.ascontiguousarray(Wp[:, C_IN:].T).astype(ml_dtypes.bfloat16)
    bmat = np.ascontiguousarray(
        np.asarray(b, dtype=np.float32)[perm].reshape(NM, 128).T
    )
    in_maps = []
    for kcore in range(N_CORES):
        xs = np.asarray(x[kcore * B_LOC : (kcore + 1) * B_LOC, :, :T], np.float32)
        xTc = np.ascontiguousarray(xs.transpose(1, 2, 0).reshape(C_IN, T * B_LOC))
        in_maps.append({"xT": xTc, "wxT": wxT, "whT": whT, "bmat": bmat})
    return in_maps


def _assemble(results, T):
    out = np.empty((B, C_OUT, T), dtype=np.float32)
    for kcore in range(N_CORES):
        yk = np.asarray(results[kcore]["y"]).astype(np.float32)  # [128, T, 32]
        out[kcore * B_LOC : (kcore + 1) * B_LOC] = (
            yk.reshape(128, T, NKH, B_LOC).transpose(3, 2, 0, 1).reshape(
                B_LOC, C_OUT, T
            )
        )
    return out


def run(x, W, b, T=T_FULL, **spmd_kwargs):
    from concourse.bass_utils import run_bass_kernel_spmd

    nc = _get_program(T)
    in_maps = _prep_inputs(x, W, b, T)
    res = run_bass_kernel_spmd(nc, in_maps, core_ids=list(range(N_CORES)), **spmd_kwargs)
    return _assemble(res.results, T), res


def kernel(x, W, b):
    out, _ = run(x, W, b, T_FULL)
    return out

